# revision 16
# baseline (speedup 1.0000x reference)
"""ParallelHyenaOperator Trainium2 kernel.

out = (irfft(rfft(u,2L) * rfft(k,2L))[:L] + u*d_bias) * x1,  u = x2*v, k = h*decay

Strategy: shard D=768 channels across 8 cores (96/core). Per channel, both
batches are packed into one complex FFT (z = u0 + i*u1); the conv theorem
gives y0 + i*y1 = ifft(fft(z) * fft(k)). The 16384-point FFT is a two-stage
radix-128 factorization where each stage is a 128x128 matmul on the tensor
engine (bf16), with pointwise twiddle/product stages on DVE and PSUM->SBUF
evacuation on the scalar engine. Pre/post gating stays fp32.

Note: all matmul operands are kept at base_partition 0 — an accumulating
matmul pair whose second operand sits at base partition 64 was observed to
hard-fault the device.
"""

import math
import os
import numpy as np
import ml_dtypes

B, D, L = 2, 768, 8192
NCORES = 8
DPC = int(os.environ.get("HYENA_DPC", D // NCORES))  # channels per core
NF = 2 * L                 # 16384 FFT size
G = 4                      # channels per group (batched free dim = 512)
NGROUPS = DPC // G
LOG_R_MIN, LOG_R_MAX = 0.0, 2.0

BF16 = ml_dtypes.bfloat16


def _make_consts():
    n2 = np.arange(64)
    n1 = np.arange(128)
    k1 = np.arange(128)
    k2 = np.arange(128)
    m = np.arange(64)

    Wc = np.exp(-2j * np.pi * np.outer(n2, k2) / 128)        # [64,128]
    T = np.exp(-2j * np.pi * np.outer(n1, k2) / NF)          # [128,128]
    W2 = np.exp(-2j * np.pi * np.outer(n1, k1) / 128)        # [128,128]
    Wcc = np.exp(+2j * np.pi * np.outer(k1, n1) / 128)       # [128,128]
    T2t = np.exp(+2j * np.pi * np.outer(k2, n1) / NF)        # [128,128] ([k2,n1])
    W2c = np.exp(+2j * np.pi * np.outer(k2, m) / 128) / NF   # [128,64]

    bf = lambda a: np.ascontiguousarray(a, dtype=np.float32).astype(BF16)
    c = {}
    c["wc_r"] = bf(Wc.real)          # [64,128]
    c["wc_i"] = bf(Wc.imag)
    c["wc_ni"] = bf(-Wc.imag)
    c["w2_r"] = bf(W2.real)
    c["w2_i"] = bf(W2.imag)
    c["w2_ni"] = bf(-W2.imag)
    c["wcc_r"] = bf(Wcc.real)
    c["wcc_i"] = bf(Wcc.imag)
    c["wcc_ni"] = bf(-Wcc.imag)
    # twiddles replicated G times along free dim
    c["t_r"] = bf(np.tile(T.real, (1, G)))
    c["t_i"] = bf(np.tile(T.imag, (1, G)))
    c["t2t_r"] = bf(np.tile(T2t.real, (1, G)))
    c["t2t_i"] = bf(np.tile(T2t.imag, (1, G)))
    # S2' weights [k2, n2] (64 cols each)
    c["w2c_r"] = bf(W2c.real)
    c["w2c_i"] = bf(W2c.imag)
    c["w2c_ni"] = bf(-W2c.imag)

    # decay = exp(-logspace(r)[d] * linspace(0,1,L)), module constant
    r = np.logspace(LOG_R_MIN, LOG_R_MAX, D).astype(np.float64)
    t = np.linspace(0.0, 1.0, L)
    decay = np.exp(-np.outer(r, t))
    c["_decay_full"] = np.ascontiguousarray(decay.astype(np.float32))
    return c


_CONSTS = _make_consts()
_NC_CACHE = {}

CONST_NAMES = ["wc_r", "wc_i", "wc_ni", "w2_r", "w2_i", "w2_ni",
               "wcc_r", "wcc_i", "wcc_ni", "t_r", "t_i", "t2t_r", "t2t_i",
               "w2c_r", "w2c_i", "w2c_ni"]


def _build_nc():
    import concourse.bacc as bacc
    import concourse.tile as tile
    from concourse import mybir

    dt = mybir.dt
    AF = mybir.AluOpType

    nc = bacc.Bacc("TRN2", target_bir_lowering=False, debug=False,
                   num_devices=NCORES)

    def din(name, shape, d):
        return nc.dram_tensor(name, shape, d, kind="ExternalInput").ap()

    x1d = din("x1s", [B, DPC, L], dt.float32)
    x2d = din("x2s", [B, DPC, L], dt.float32)
    vd = din("vs", [B, DPC, L], dt.float32)
    hd = din("hs", [DPC, L], dt.float32)
    dbd = din("db_bc", [128, DPC], dt.float32)
    decd = din("decays", [DPC, L], dt.float32)
    cc = {}
    for nm in CONST_NAMES:
        shp = list(_CONSTS[nm].shape)
        cc[nm] = din(nm, shp, dt.bfloat16)
    outd = nc.dram_tensor("out", [B, DPC, L], dt.float32,
                          kind="ExternalOutput").ap()

    FW = 128 * G  # group free width

    with tile.TileContext(nc, trace_sim=False) as tc:
        cpool = tc.alloc_tile_pool(name="consts", bufs=1)
        iopool = tc.alloc_tile_pool(name="io", bufs=2 * G)
        upool = tc.alloc_tile_pool(name="u", bufs=3 * G)
        gpool = tc.alloc_tile_pool(name="grp", bufs=2)
        pspool = tc.alloc_tile_pool(name="ps", bufs=7, space="PSUM")

        csb = {}
        for nm, ap in cc.items():
            t = cpool.tile(list(ap.shape), dt.bfloat16, tag=nm)
            nc.sync.dma_start(t[:], ap)
            csb[nm] = t
        dbt = cpool.tile([128, DPC], dt.float32, tag="dbt")
        nc.sync.dma_start(dbt[:], dbd)

        def cmul(eng, out_r, out_i, a_r, a_i, b_r, b_i, tmp_pool, fw):
            # (out_r + i*out_i) = (a_r + i*a_i) * (b_r + i*b_i), bf16
            m1 = tmp_pool.tile([128, fw], dt.bfloat16, tag="cm_m1")
            m2 = tmp_pool.tile([128, fw], dt.bfloat16, tag="cm_m2")
            eng.tensor_tensor(m1[:], a_r[:], b_r[:], AF.mult)
            eng.tensor_tensor(m2[:], a_i[:], b_i[:], AF.mult)
            eng.tensor_tensor(out_r[:], m1[:], m2[:], AF.subtract)
            m3 = tmp_pool.tile([128, fw], dt.bfloat16, tag="cm_m1")
            m4 = tmp_pool.tile([128, fw], dt.bfloat16, tag="cm_m2")
            eng.tensor_tensor(m3[:], a_r[:], b_i[:], AF.mult)
            eng.tensor_tensor(m4[:], a_i[:], b_r[:], AF.mult)
            eng.tensor_tensor(out_i[:], m3[:], m4[:], AF.add)

        for g in range(NGROUPS):
            chans = [g * G + j for j in range(G)]
            uts, x1ts = [], []
            # S1 psum accumulators, one [128,128] slice per channel
            z0r = pspool.tile([128, FW], dt.float32, tag="ps")
            z0i = pspool.tile([128, FW], dt.float32, tag="ps")
            zk0r = pspool.tile([128, FW], dt.float32, tag="ps")
            zk0i = pspool.tile([128, FW], dt.float32, tag="ps")
            for j, c in enumerate(chans):
                # [64, 256] layout: batch b occupies free cols [128b, 128b+128)
                x2t = iopool.tile([64, 256], dt.float32, tag="x2")
                vt = iopool.tile([64, 256], dt.float32, tag="v")
                x1t = iopool.tile([64, 256], dt.float32, tag="x1")
                ht = iopool.tile([64, 128], dt.float32, tag="h")
                dct = iopool.tile([64, 128], dt.float32, tag="dec")
                for bb in range(2):
                    fs = slice(128 * bb, 128 * (bb + 1))
                    nc.sync.dma_start(
                        x2t[:, fs],
                        x2d[bb, c, :].rearrange("(p q) -> p q", p=64))
                    nc.sync.dma_start(
                        vt[:, fs],
                        vd[bb, c, :].rearrange("(p q) -> p q", p=64))
                    nc.sync.dma_start(
                        x1t[:, fs],
                        x1d[bb, c, :].rearrange("(p q) -> p q", p=64))
                nc.sync.dma_start(
                    ht[:], hd[c, :].rearrange("(p q) -> p q", p=64))
                nc.sync.dma_start(
                    dct[:], decd[c, :].rearrange("(p q) -> p q", p=64))

                ut = upool.tile([64, 256], dt.float32, tag="u")
                nc.vector.tensor_tensor(ut[:], x2t[:], vt[:], AF.mult)
                zt = upool.tile([64, 256], dt.bfloat16, tag="z")
                nc.vector.tensor_copy(zt[:], ut[:])
                kbt = upool.tile([64, 128], dt.bfloat16, tag="kb")
                nc.vector.tensor_tensor(kbt[:], ht[:], dct[:], AF.mult)
                uts.append(ut)
                x1ts.append(x1t)

                sl = slice(j * 128, (j + 1) * 128)
                zr = zt[:, 0:128]
                zi = zt[:, 128:256]
                wr, wi, wni = csb["wc_r"], csb["wc_i"], csb["wc_ni"]
                # S1 (u): Z0[n1,k2] = sum_n2 z[n2,n1] * Wc[n2,k2]
                nc.tensor.matmul(z0r[:, sl], zr, wr[:], start=True, stop=False)
                nc.tensor.matmul(z0r[:, sl], zi, wni[:], start=False, stop=True)
                nc.tensor.matmul(z0i[:, sl], zr, wi[:], start=True, stop=False)
                nc.tensor.matmul(z0i[:, sl], zi, wr[:], start=False, stop=True)
                # S1 (k): real input
                nc.tensor.matmul(zk0r[:, sl], kbt[:], wr[:], start=True, stop=True)
                nc.tensor.matmul(zk0i[:, sl], kbt[:], wi[:], start=True, stop=True)

            # evacuate S1 psum -> bf16 sbuf (scalar engine)
            z0rb = gpool.tile([128, FW], dt.bfloat16, tag="z0rb")
            z0ib = gpool.tile([128, FW], dt.bfloat16, tag="z0ib")
            zk0rb = gpool.tile([128, FW], dt.bfloat16, tag="zk0rb")
            zk0ib = gpool.tile([128, FW], dt.bfloat16, tag="zk0ib")
            nc.scalar.copy(z0rb[:], z0r[:])
            nc.scalar.copy(z0ib[:], z0i[:])
            nc.scalar.copy(zk0rb[:], zk0r[:])
            nc.scalar.copy(zk0ib[:], zk0i[:])

            # forward twiddle
            z1r = gpool.tile([128, FW], dt.bfloat16, tag="z1r")
            z1i = gpool.tile([128, FW], dt.bfloat16, tag="z1i")
            cmul(nc.vector, z1r, z1i, z0rb, z0ib, csb["t_r"], csb["t_i"],
                 gpool, FW)
            zk1r = gpool.tile([128, FW], dt.bfloat16, tag="zk1r")
            zk1i = gpool.tile([128, FW], dt.bfloat16, tag="zk1i")
            cmul(nc.vector, zk1r, zk1i, zk0rb, zk0ib, csb["t_r"], csb["t_i"],
                 gpool, FW)

            # S2: P[k1,k2] = sum_n1 W2[n1,k1] * Z1[n1,k2], grouped N=512
            pzr = pspool.tile([128, FW], dt.float32, tag="ps")
            pzi = pspool.tile([128, FW], dt.float32, tag="ps")
            pkr = pspool.tile([128, FW], dt.float32, tag="ps")
            pki = pspool.tile([128, FW], dt.float32, tag="ps")
            w2r, w2i, w2ni = csb["w2_r"], csb["w2_i"], csb["w2_ni"]
            nc.tensor.matmul(pzr[:], w2r[:], z1r[:], start=True, stop=False)
            nc.tensor.matmul(pzi[:], w2i[:], z1r[:], start=True, stop=False)
            nc.tensor.matmul(pkr[:], w2r[:], zk1r[:], start=True, stop=False)
            nc.tensor.matmul(pki[:], w2i[:], zk1r[:], start=True, stop=False)
            nc.tensor.matmul(pzr[:], w2ni[:], z1i[:], start=False, stop=True)
            nc.tensor.matmul(pkr[:], w2ni[:], zk1i[:], start=False, stop=True)
            nc.tensor.matmul(pzi[:], w2r[:], z1i[:], start=False, stop=True)
            nc.tensor.matmul(pki[:], w2r[:], zk1i[:], start=False, stop=True)

            # evacuate P psum -> bf16
            pzrb = gpool.tile([128, FW], dt.bfloat16, tag="pzrb")
            pzib = gpool.tile([128, FW], dt.bfloat16, tag="pzib")
            pkrb = gpool.tile([128, FW], dt.bfloat16, tag="pkrb")
            pkib = gpool.tile([128, FW], dt.bfloat16, tag="pkib")
            nc.scalar.copy(pzrb[:], pzr[:])
            nc.scalar.copy(pzib[:], pzi[:])
            nc.scalar.copy(pkrb[:], pkr[:])
            nc.scalar.copy(pkib[:], pki[:])

            # spectral product
            pyr = gpool.tile([128, FW], dt.bfloat16, tag="pyr")
            pyi = gpool.tile([128, FW], dt.bfloat16, tag="pyi")
            cmul(nc.vector, pyr, pyi, pzrb, pzib, pkrb, pkib, gpool, FW)

            # S1': At[k2,n1] = sum_k1 P_y[k1,k2] * Wcc[k1,n1], per-channel lhsT
            atr = pspool.tile([128, FW], dt.float32, tag="ps")
            ati = pspool.tile([128, FW], dt.float32, tag="ps")
            for j in range(G):
                sl = slice(j * 128, (j + 1) * 128)
                pr = pyr[:, sl]
                pi = pyi[:, sl]
                nc.tensor.matmul(atr[:, sl], pr, csb["wcc_r"][:], start=True, stop=False)
                nc.tensor.matmul(ati[:, sl], pr, csb["wcc_i"][:], start=True, stop=False)
                nc.tensor.matmul(atr[:, sl], pi, csb["wcc_ni"][:], start=False, stop=True)
                nc.tensor.matmul(ati[:, sl], pi, csb["wcc_r"][:], start=False, stop=True)

            atrb = gpool.tile([128, FW], dt.bfloat16, tag="atrb")
            atib = gpool.tile([128, FW], dt.bfloat16, tag="atib")
            nc.scalar.copy(atrb[:], atr[:])
            nc.scalar.copy(atib[:], ati[:])

            # inverse twiddle
            btr = gpool.tile([128, FW], dt.bfloat16, tag="btr")
            bti = gpool.tile([128, FW], dt.bfloat16, tag="bti")
            cmul(nc.vector, btr, bti, atrb, atib, csb["t2t_r"], csb["t2t_i"],
                 gpool, FW)

            # S2': y_b[n2,n1] = Re/Im of sum_k2 W2c[k2,n2] * Bt[k2,n1], grouped
            yg0 = pspool.tile([64, FW], dt.float32, tag="ps")   # Re -> batch 0
            yg1 = pspool.tile([64, FW], dt.float32, tag="ps")   # Im -> batch 1
            nc.tensor.matmul(yg0[:], csb["w2c_r"][:], btr[:], start=True, stop=False)
            nc.tensor.matmul(yg1[:], csb["w2c_i"][:], btr[:], start=True, stop=False)
            nc.tensor.matmul(yg0[:], csb["w2c_ni"][:], bti[:], start=False, stop=True)
            nc.tensor.matmul(yg1[:], csb["w2c_r"][:], bti[:], start=False, stop=True)

            # post: out_b = (y_b + db*u_b) * x1_b   (fp32)
            for j, c in enumerate(chans):
                sl = slice(j * 128, (j + 1) * 128)
                for bb, yg in enumerate((yg0, yg1)):
                    fs = slice(128 * bb, 128 * (bb + 1))
                    tt = upool.tile([64, 128], dt.float32, tag="t")
                    nc.vector.scalar_tensor_tensor(
                        tt[:], uts[j][:, fs], dbt[0:64, c:c + 1], yg[:, sl],
                        AF.mult, AF.add)
                    ot = upool.tile([64, 128], dt.float32, tag="o")
                    nc.vector.tensor_tensor(ot[:], tt[:], x1ts[j][:, fs],
                                            AF.mult)
                    nc.sync.dma_start(
                        outd[bb, c, :].rearrange("(p q) -> p q", p=64), ot[:])

        for p in (pspool, gpool, upool, iopool, cpool):
            p.release()

    nc.compile()
    return nc


def _get_nc():
    if "nc" not in _NC_CACHE:
        _NC_CACHE["nc"] = _build_nc()
    return _NC_CACHE["nc"]


def make_in_maps(x1, x2, v, h, d_bias):
    c = _CONSTS
    in_maps = []
    for core in range(NCORES):
        sl = slice(core * DPC, (core + 1) * DPC)
        m = {
            "x1s": np.ascontiguousarray(x1[:, sl]),
            "x2s": np.ascontiguousarray(x2[:, sl]),
            "vs": np.ascontiguousarray(v[:, sl]),
            "hs": np.ascontiguousarray(h[sl]),
            "db_bc": np.ascontiguousarray(
                np.broadcast_to(d_bias[sl][None, :], (128, DPC))),
            "decays": np.ascontiguousarray(c["_decay_full"][sl]),
        }
        for nm in CONST_NAMES:
            m[nm] = c[nm]
        in_maps.append(m)
    return in_maps


def kernel(x1, x2, v, h, d_bias):
    from concourse import bass_utils

    x1 = np.ascontiguousarray(x1, dtype=np.float32)
    x2 = np.ascontiguousarray(x2, dtype=np.float32)
    v = np.ascontiguousarray(v, dtype=np.float32)
    h = np.ascontiguousarray(h, dtype=np.float32)
    d_bias = np.ascontiguousarray(d_bias, dtype=np.float32)

    nc = _get_nc()
    in_maps = make_in_maps(x1, x2, v, h, d_bias)
    res = bass_utils.run_bass_kernel_spmd(
        nc, in_maps, core_ids=list(range(NCORES)))
    out = np.concatenate([r["out"] for r in res.results], axis=1)
    return out.astype(np.float32)


if __name__ == "__main__":
    rng = np.random.default_rng(0)
    inputs = {
        "x1": rng.standard_normal((B, D, L)).astype(np.float32),
        "x2": rng.standard_normal((B, D, L)).astype(np.float32),
        "v": rng.standard_normal((B, D, L)).astype(np.float32),
        "h": (rng.standard_normal((D, L)) / math.sqrt(L) * 1e-5).astype(np.float32),
        "d_bias": rng.standard_normal(D).astype(np.float32),
    }
    out = kernel(**inputs)
    print(out.shape, out.dtype)
